# revision 1
# baseline (speedup 1.0000x reference)
"""Trainium2 Bass kernel for nn_MultiHeadAttention_7584912245188.

Reference computes (no softmax!):
    qkv = x @ Wqkv + bqkv ; split q,k,v ; per head: y = (q k^T / sqrt(D)) v
    out = y @ Wff + bff

Because there is no softmax, attention is linear and reassociates:
    (Q K^T) V = Q (K^T V).
With X_aug = [X | 1] ([N, 97]) and G = X_aug^T X_aug ([97, 97]), the whole
module collapses (associativity, per head h):
    out = X_aug @ Wfin,   Wfin = sum_h P_h G Q_h + e_last bff^T
    P_h = Wq_aug_h Wk_aug_h^T [97,97],  Q_h = D^-0.5 Wv_aug_h Wff_h [97,96]
P_h / Q_h are host-precomputed from the weights. On device per batch:
    G (16 accumulating matmuls over row chunks)
    R = G @ [Q_0|...|Q_5]            (2 matmuls, free dim 288)
    Wfin = sum_h P_h R_h + bff term  (7 PSUM-accumulating matmuls, no hops)
    out chunks = X_chunk @ Wfin      (via PE-transposed X chunks)
O(N*E^2) instead of O(N^2*D).

Precision: x is cast to fp16 on the host (halves HBM load traffic); all
matmul operands are fp16 (exact products, f32 PSUM accumulate); Gram
partials accumulate in f32; the output is written fp16 and upcast on the
host. Measured end-to-end rel err ~6e-4.

Sharding (8 cores): core c -> (batch b = c//2, sequence half h = c%2).
Each core receives x[b] (with a ones column appended host-side) rolled so
"its" half comes first, computes G from the full batch (redundantly within
the pair - cheaper than a collective), and writes only its half of the
output rows.

Row layout: x is processed as 2 halves of 1024 rows; within a half,
row = 1024*h + 8*p + j (p = SBUF partition, j = 0..7), so each half is
ONE fully contiguous ~1.5KB segment per partition per DMA (the cost is
dominated by descriptor count, not bytes). Chunks c = 8*h + j are
arbitrary row groups: G sums over all rows regardless of grouping, and
the transpose/final/output steps use the same mapping consistently.

DMA plan: HWDGE DMAs serialize on shared HW lanes (~0.6us slot per
~100KB chunk) and each DMA pays ~0.9us completion latency, so x goes as
4 quarter DMAs (own tile each - readers wait only on their quarter) on
the two HWDGE rings, the packed weights ride the separate SWDGE path,
and the Gram matmuls pipeline behind the quarter arrivals. The transpose
identity is built on-chip by gpsimd before the weights DMA.
"""

import numpy as np
from contextlib import ExitStack

import concourse.bass as bass
import concourse.tile as tile
from concourse import bacc, mybir
from concourse import bass_utils
from concourse.masks import make_identity

B, N, E = 4, 2048, 96
H = 6
D = E // H            # 16
P = 128
NCH = N // P          # 16 chunks of 128 rows
HALF = NCH // 2       # 8 chunks per core
J = 4                 # rows per partition per quarter
EA = E + 1            # 97 (augmented with ones column)
SCALE = float(D) ** -0.5
F32 = mybir.dt.float32
F16 = mybir.dt.float16

# wpack (fp16, 97 partitions) column layout: PcatT | Qcat | onehot | bff
C_P = 0
C_Q = C_P + H * EA           # 582
C_OH = C_Q + H * E           # 1158
C_BF = C_OH + EA             # 1255
WPACK_COLS = C_BF + E        # 1351

N_CORES = 8

_NC_CACHE = {}
LAST_RESULTS = None


def _build_nc():
    nc = bacc.Bacc(
        "TRN2", target_bir_lowering=False, debug=False, num_devices=N_CORES
    )
    x = nc.dram_tensor("x", [N, EA], F16, kind="ExternalInput").ap()
    wpacki = nc.dram_tensor("wpack", [EA, WPACK_COLS], F16, kind="ExternalInput").ap()
    out = nc.dram_tensor("out", [N // 2, E], F16, kind="ExternalOutput").ap()

    with tile.TileContext(nc) as tc, ExitStack() as ctx:
        consts = ctx.enter_context(tc.tile_pool(name="consts", bufs=1))
        big = ctx.enter_context(tc.tile_pool(name="big", bufs=1))
        small = ctx.enter_context(tc.tile_pool(name="small", bufs=1))
        outp = ctx.enter_context(tc.tile_pool(name="outp", bufs=1))
        ps_t = ctx.enter_context(tc.tile_pool(name="ps_t", bufs=2, space="PSUM"))
        ps_g = ctx.enter_context(tc.tile_pool(name="ps_g", bufs=1, space="PSUM"))
        ps_r = ctx.enter_context(tc.tile_pool(name="ps_r", bufs=2, space="PSUM"))
        ps_w = ctx.enter_context(tc.tile_pool(name="ps_w", bufs=1, space="PSUM"))
        ps_o = ctx.enter_context(tc.tile_pool(name="ps_o", bufs=2, space="PSUM"))

        # --- loads: identity+weights via gpsimd/SWDGE (own path), x as two
        # half DMAs on the two HWDGE rings (one tile per DMA)
        xh = x.rearrange("(h p j) e -> h p j e", h=2, j=HALF)
        XA = big.tile([P, HALF, EA], F16)
        nc.sync.dma_start(out=XA[:], in_=xh[0])
        XB = big.tile([P, HALF, EA], F16)
        nc.gpsimd.dma_start(out=XB[:], in_=xh[1])                 # SWDGE
        wp = consts.tile([EA, WPACK_COLS], F16)
        nc.gpsimd.dma_start(out=wp[:], in_=wpacki)                # SWDGE
        id_sb = consts.tile([P, P], F16)
        make_identity(nc, id_sb[:])                               # gpsimd


        def Xc(c):
            return XA[:, c, :] if c < HALF else XB[:, c - HALF, :]

        # --- G = X_aug^T X_aug: one 16-matmul PSUM accumulation group,
        # half B pipelining behind its DMA
        g_ps = ps_g.tile([EA, EA], F32)
        for c in range(NCH):
            nc.tensor.matmul(
                g_ps[:], lhsT=Xc(c), rhs=Xc(c),
                start=(c == 0), stop=(c == NCH - 1),
            )
        g_h = small.tile([EA, EA], F16)
        nc.vector.tensor_copy(out=g_h[:], in_=g_ps[:])

        # --- chain with transposes interleaved into its latency gaps.
        # R = G @ Qcat (2 matmuls, free 288) staged to fp16; the 8 PE
        # transposes of my half run while DVE casts G / stages R.
        XT = big.tile([EA, HALF, P], F16)
        r_h = small.tile([EA, H * E], F16)
        pt0 = ps_t.tile([EA, J, P], F16, tag="pt", name="pt0")
        for j in range(J):
            nc.tensor.transpose(out=pt0[:, j, :], in_=Xc(j), identity=id_sb[:])
        nc.vector.tensor_copy(out=XT[:, 0:J, :], in_=pt0[:])
        r0 = ps_r.tile([EA, H * E // 2], F32, tag="r", name="r0")
        nc.tensor.matmul(
            r0[:], lhsT=g_h[:], rhs=wp[:, C_Q : C_Q + H * E // 2],
            start=True, stop=True,
        )
        nc.vector.tensor_copy(out=r_h[:, 0 : H * E // 2], in_=r0[:])
        pt1 = ps_t.tile([EA, J, P], F16, tag="pt", name="pt1")
        for j in range(J):
            nc.tensor.transpose(
                out=pt1[:, j, :], in_=Xc(J + j), identity=id_sb[:]
            )
        nc.vector.tensor_copy(out=XT[:, J : 2 * J, :], in_=pt1[:])
        r1 = ps_r.tile([EA, H * E // 2], F32, tag="r", name="r1")
        nc.tensor.matmul(
            r1[:], lhsT=g_h[:], rhs=wp[:, C_Q + H * E // 2 : C_Q + H * E],
            start=True, stop=True,
        )
        nc.vector.tensor_copy(out=r_h[:, H * E // 2 : H * E], in_=r1[:])

        # --- Wfin = sum_h P_h R_h + e_last bff^T  (one PSUM accum group)
        wf_ps = ps_w.tile([EA, E], F32)
        for h in range(H):
            nc.tensor.matmul(
                wf_ps[:],
                lhsT=wp[:, C_P + h * EA : C_P + (h + 1) * EA],
                rhs=r_h[:, h * E : (h + 1) * E],
                start=(h == 0),
                stop=False,
            )
        nc.tensor.matmul(
            wf_ps[:],
            lhsT=wp[0:1, C_OH : C_OH + EA],
            rhs=wp[0:1, C_BF : C_BF + E],
            start=False,
            stop=True,
        )
        wf_h = small.tile([EA, E], F16)
        nc.vector.tensor_copy(out=wf_h[:], in_=wf_ps[:])

        # --- finals: out chunk = X_chunk @ Wfin via lhsT = XT chunk
        osb = outp.tile([P, HALF, E], F16)
        for grp in range(2):
            og = ps_o.tile([P, J, E], F32, tag="og", name=f"og{grp}")
            for j in range(J):
                nc.tensor.matmul(
                    og[:, j, :], lhsT=XT[:, J * grp + j, :], rhs=wf_h[:],
                    start=True, stop=True,
                )
            nc.vector.tensor_copy(
                out=osb[:, J * grp : J * (grp + 1), :], in_=og[:]
            )
        nc.sync.dma_start(
            out=out.rearrange("(p j) e -> p j e", j=HALF), in_=osb[:]
        )

    nc.compile()
    return nc


def get_nc():
    if "nc" not in _NC_CACHE:
        _NC_CACHE["nc"] = _build_nc()
    return _NC_CACHE["nc"]


def _host_weights(Wqkv, bqkv, Wff, bff):
    waug = np.concatenate(
        [np.asarray(Wqkv, np.float64), np.asarray(bqkv, np.float64)[None, :]], axis=0
    )
    Wq, Wk, Wv = waug[:, 0:E], waug[:, E : 2 * E], waug[:, 2 * E : 3 * E]
    Wff = np.asarray(Wff, np.float64)
    wp = np.zeros((EA, WPACK_COLS), np.float16)
    for h in range(H):
        hd = slice(h * D, (h + 1) * D)
        Ph = Wq[:, hd] @ Wk[:, hd].T                    # [97, 97]
        Qh = SCALE * (Wv[:, hd] @ Wff[hd, :])           # [97, 96]
        wp[0:EA, C_P + h * EA : C_P + (h + 1) * EA] = Ph.T.astype(np.float16)
        wp[0:EA, C_Q + h * E : C_Q + (h + 1) * E] = Qh.astype(np.float16)
    wp[0, C_OH + E] = 1.0                               # e_last selector row
    wp[0, C_BF : C_BF + E] = np.asarray(bff, np.float16)
    return {"wpack": wp}


def make_in_maps(x, Wqkv, bqkv, Wff, bff):
    x = np.asarray(x, np.float32)
    w = _host_weights(Wqkv, bqkv, Wff, bff)
    ones = np.ones((N, 1), np.float16)
    x16 = x.astype(np.float16)
    in_maps = []
    for c in range(N_CORES):
        b, h = divmod(c, 2)
        xb = x16[b]
        if h:
            xb = np.concatenate([xb[N // 2 :], xb[: N // 2]], axis=0)
        m = {"x": np.ascontiguousarray(np.concatenate([xb, ones], axis=1))}
        m.update(w)
        in_maps.append(m)
    return in_maps


def assemble(results):
    out = np.empty((B, N, E), np.float32)
    for c in range(N_CORES):
        b, h = divmod(c, 2)
        out[b, h * (N // 2) : (h + 1) * (N // 2)] = results[c]["out"]
    return out


def kernel(x, Wqkv, bqkv, Wff, bff):
    global LAST_RESULTS
    nc = get_nc()
    in_maps = make_in_maps(x, Wqkv, bqkv, Wff, bff)
    res = bass_utils.run_bass_kernel_spmd(
        nc, in_maps, core_ids=list(range(N_CORES))
    )
    LAST_RESULTS = res
    return assemble(res.results)



# revision 32
# speedup vs baseline: 1.0143x; 1.0143x over previous
"""Trainium2 Bass kernel for nn_MultiHeadAttention_7584912245188.

Reference computes (no softmax!):
    qkv = x @ Wqkv + bqkv ; split q,k,v ; per head: y = (q k^T / sqrt(D)) v
    out = y @ Wff + bff

No softmax => attention is linear and reassociates: (Q K^T) V = Q (K^T V).
With X_aug = [X | 1] ([N, 97]) and G = X_aug^T X_aug ([97, 97]) the module
collapses to out = X_aug @ Wfin computed on device as:
    V = G @ Wk_aug                                  [97, 96]  (1 matmul)
    Call_h = V_h^T @ Wvff_h                         [16, 96]  (6 matmuls,
             head blocks along the free dim; Wvff_h = D^-0.5 Wv_aug_h Wff_h)
    Wfin = sum_h Wq_aug_h @ Call_h + e_last bff^T   [97, 96]  (7-matmul group)
    out rows {8p+j} = X_chunk @ Wfin                (8 matmuls via transposed X)
O(N*E^2) instead of O(N^2*D).

Sharding (8 cores): core c -> (batch b = c//2, half h = c%2). Each core
computes the full-batch Gram redundantly (cheaper than a collective) and
writes its own half of the rows.

Schedule (fixed DMA costs dominate: HWDGE issue ~630 + 650 DGE delay, SWDGE
prep ~1040 + 650, 900ns completion sem, shared ~360GB/s DMA engines):
  - near half rides the first sync-HWDGE slot in fp16 (Gram + transposes);
    the far half is QUANTIZED TO FP8 (it only feeds the Gram; G is
    diagonally dominated, measured end-to-end rel err ~6e-3) and rides the
    Pool SWDGE path whose descriptor prep overlaps the first transfer.
  - folded weights are ~160KB (vs 262KB for the P/Q form) on the two
    activation-queue HWDGE slots, ordered Wk|Wvff|identity first.
  - a chain of tiny matmuls warms the PE p-state ramp before the real work.
  - X^T comes from 8 PE transposes placed in PE idle gaps; the PSUM->SBUF
    staging copies run on the Activation engine, off the critical chain.
  - outputs leave as two HWDGE stores (sync + activation queues) so the
    second issue/delay overlaps the first transfer.
"""

import numpy as np
from contextlib import ExitStack

import ml_dtypes
import concourse.bass as bass
import concourse.tile as tile
from concourse import bacc, mybir
from concourse import bass_utils

B, N, E = 4, 2048, 96
H = 6
D = E // H            # 16
EA = E + 1            # 97 (augmented ones column)
NH = N // 2           # 1024 rows per half
NCH = 8               # row chunks per half (chunk j = rows {8p + j})
SCALE = float(D) ** -0.5
F32 = mybir.dt.float32
F16 = mybir.dt.float16
F8 = mybir.dt.float8e4
NP_F8 = ml_dtypes.float8_e4m3

N_WARM = 12           # PE p-state warmup matmuls
WARM_COLS = 128

# wpack (fp16, 97 partitions) column layout: Wk_aug | Wvff | identity
C_WK = 0              # Wk_aug [97, 96]
C_WVFF = 96           # Wvff (6 heads x [97, 96], scale folded)
C_ID = 672            # [97, 128] f16 identity (PE transpose operand)
WCOLS = 800
# wq2 (fp16, 16 partitions): per-head Wq_aug_h^T [16, 97] | onehot | bff
C_OH = 582            # [1, 97] onehot row (1.0 at col 96) - bff placement
C_BF = 679            # [1, 96] bff row
WQCOLS = 775

N_CORES = 8

_NC_CACHE = {}
LAST_RESULTS = None


def _build_nc():
    nc = bacc.Bacc(
        "TRN2", target_bir_lowering=False, debug=False, num_devices=N_CORES,
    )
    xa = nc.dram_tensor("xa", [NH, EA], F16, kind="ExternalInput").ap()
    xb = nc.dram_tensor("xb", [NH, EA], F8, kind="ExternalInput").ap()
    wpi = nc.dram_tensor("wpack", [128, WCOLS], F16, kind="ExternalInput").ap()
    wqi = nc.dram_tensor("wq2", [D, WQCOLS], F16, kind="ExternalInput").ap()
    out0 = nc.dram_tensor("out0", [128, 4 * E], F16, kind="ExternalOutput").ap()
    out1 = nc.dram_tensor("out1", [128, 4 * E], F16, kind="ExternalOutput").ap()

    with tile.TileContext(nc) as tc, ExitStack() as ctx:
        consts = ctx.enter_context(tc.tile_pool(name="consts", bufs=1))
        big = ctx.enter_context(tc.tile_pool(name="big", bufs=1))
        small = ctx.enter_context(tc.tile_pool(name="small", bufs=1))
        outp = ctx.enter_context(tc.tile_pool(name="outp", bufs=1))
        ps_gw = ctx.enter_context(tc.tile_pool(name="ps_gw", bufs=1, space="PSUM"))
        ps_v = ctx.enter_context(tc.tile_pool(name="ps_v", bufs=1, space="PSUM"))
        ps_c = ctx.enter_context(tc.tile_pool(name="ps_c", bufs=2, space="PSUM"))
        ps_t = ctx.enter_context(tc.tile_pool(name="ps_t", bufs=2, space="PSUM"))
        ps_o = ctx.enter_context(tc.tile_pool(name="ps_o", bufs=2, space="PSUM"))

        # --- near half fp16 on the first sync-HWDGE slot; far half fp8 on
        # the Pool SWDGE path (its prep overlaps xa's transfer); weights on
        # the activation HWDGE queue, Wk|Wvff|identity first
        XA = big.tile([128, NCH, EA], F16)
        nc.sync.dma_start(out=XA[:], in_=xa.rearrange("(p j) e -> p j e", j=NCH))
        XB = big.tile([128, NCH, EA], F8)
        nc.gpsimd.dma_start(out=XB[:], in_=xb.rearrange("(p j) e -> p j e", j=NCH))
        wp = consts.tile([128, WCOLS], F16)
        nc.scalar.dma_start(out=wp[:], in_=wpi)
        wq2 = consts.tile([D, WQCOLS], F16)
        nc.scalar.dma_start(out=wq2[:], in_=wqi)

        # --- PE p-state warmup: keep the tensor engine busy from ~0.7us so
        # the ramp model is past the slow state when the real matmuls start
        wu = small.tile([1, WARM_COLS], F16)
        nc.vector.memset(wu[:], 0.0)
        wu_ps = ps_o.tile([1, WARM_COLS], F32, tag="og", name="warm")
        for _ in range(N_WARM):
            nc.tensor.matmul(
                wu_ps[:], lhsT=wu[0:1, 0:1], rhs=wu[:], start=True, stop=True
            )

        # --- G = X_aug^T X_aug, one 16-matmul PSUM accumulation group
        # (near half first - it arrives first)
        g_ps = ps_gw.tile([EA, EA], F32, tag="gw", name="g")
        for c in range(NCH):
            xc = XA[:, c, :]
            nc.tensor.matmul(g_ps[:], lhsT=xc, rhs=xc, start=(c == 0), stop=False)
        for c in range(NCH):
            xc = XB[:, c, :]
            nc.tensor.matmul(
                g_ps[:], lhsT=xc, rhs=xc, start=False, stop=(c == NCH - 1)
            )
        g_h = small.tile([EA, EA], F16)
        nc.vector.tensor_copy(out=g_h[:], in_=g_ps[:])

        # --- 8 PE transposes of the near half (PE is idle while the chain
        # copies run); PSUM->SBUF staging on Act, off the critical chain
        XT = big.tile([EA, NCH, 128], F16)
        pts = []
        for grp in range(2):
            pt = ps_t.tile([EA, 4, 128], F16, tag="pt", name=f"pt{grp}")
            for j in range(4):
                nc.tensor.transpose(
                    out=pt[:, j, :], in_=XA[:, 4 * grp + j, :],
                    identity=wp[:, C_ID : C_ID + 128],
                )
            pts.append(pt)

        # --- V = G @ Wk_aug
        v_ps = ps_v.tile([EA, E], F32)
        nc.tensor.matmul(
            v_ps[:], lhsT=g_h[:], rhs=wp[0:EA, C_WK : C_WK + E], start=True, stop=True
        )
        v_h = small.tile([EA, E], F16)
        nc.vector.tensor_copy(out=v_h[:], in_=v_ps[:])
        for grp in range(2):
            nc.scalar.copy(
                out=XT[:, 4 * grp : 4 * (grp + 1), :], in_=pts[grp][:]
            )

        # --- Call[0:16, 96h:96h+96] = V_h^T Wvff_h (PSUM base-partition rule
        # forces head blocks onto the free dim; two banks, DVE + Act copies)
        call_sb = small.tile([D, H * E], F16)
        for half in range(2):
            ca_ps = ps_c.tile([D, 3 * E], F32, tag="call", name=f"call{half}")
            for hh in range(3):
                h = 3 * half + hh
                nc.tensor.matmul(
                    ca_ps[:, E * hh : E * (hh + 1)],
                    lhsT=v_h[:, D * h : D * (h + 1)],
                    rhs=wp[0:EA, C_WVFF + E * h : C_WVFF + E * (h + 1)],
                    start=True, stop=True,
                )
            cp = nc.vector.tensor_copy if half == 0 else nc.scalar.copy
            cp(out=call_sb[:, 3 * E * half : 3 * E * (half + 1)], in_=ca_ps[:])

        # --- Wfin = sum_h Wq_aug_h @ Call_h + e_last bff^T (one accum group,
        # PSUM bank shared with G - dead after g_h)
        wf_ps = ps_gw.tile([EA, E], F32, tag="gw", name="wf")
        for h in range(H):
            nc.tensor.matmul(
                wf_ps[:],
                lhsT=wq2[:, EA * h : EA * (h + 1)],
                rhs=call_sb[:, E * h : E * (h + 1)],
                start=(h == 0), stop=False,
            )
        nc.tensor.matmul(
            wf_ps[:],
            lhsT=wq2[0:1, C_OH : C_OH + EA],
            rhs=wq2[0:1, C_BF : C_BF + E],
            start=False, stop=True,
        )
        wf_h = small.tile([EA, E], F16)
        nc.vector.tensor_copy(out=wf_h[:], in_=wf_ps[:])

        # --- finals: out rows {8p+j} = X_chunk @ Wfin; each half leaves as
        # its own HWDGE store so the issues/transfers overlap
        osb = outp.tile([128, 2, 4 * E], F16)
        for g in range(2):
            og = ps_o.tile([128, 4, E], F32, tag="og", name=f"og{g}")
            for j4 in range(4):
                nc.tensor.matmul(
                    og[:, j4, :], lhsT=XT[:, 4 * g + j4, :], rhs=wf_h[:],
                    start=True, stop=True,
                )
            cp = nc.vector.tensor_copy if g == 0 else nc.scalar.copy
            cp(out=osb[:, g, :], in_=og[:].rearrange("p a b -> p (a b)"))
            dma = nc.sync.dma_start if g == 0 else nc.scalar.dma_start
            dma(out=(out0 if g == 0 else out1), in_=osb[:, g, :])

    nc.compile()
    return nc


def get_nc():
    if "nc" not in _NC_CACHE:
        _NC_CACHE["nc"] = _build_nc()
    return _NC_CACHE["nc"]


def _host_weights(Wqkv, bqkv, Wff, bff):
    waug = np.concatenate(
        [np.asarray(Wqkv, np.float64), np.asarray(bqkv, np.float64)[None, :]], axis=0
    )
    Wq, Wk, Wv = waug[:, 0:E], waug[:, E : 2 * E], waug[:, 2 * E : 3 * E]
    Wff = np.asarray(Wff, np.float64)
    wp = np.zeros((128, WCOLS), np.float16)
    wp[0:EA, C_WK : C_WK + E] = Wk.astype(np.float16)
    wp[:, C_ID : C_ID + 128] = np.eye(128, dtype=np.float16)
    wq2 = np.zeros((D, WQCOLS), np.float16)
    for h in range(H):
        hd = slice(h * D, (h + 1) * D)
        wp[0:EA, C_WVFF + E * h : C_WVFF + E * (h + 1)] = (
            SCALE * (Wv[:, hd] @ Wff[hd, :])
        ).astype(np.float16)
        wq2[:, EA * h : EA * (h + 1)] = Wq[:, hd].T.astype(np.float16)
    wq2[0, C_OH + E] = 1.0
    wq2[0, C_BF : C_BF + E] = np.asarray(bff, np.float16)
    return {"wpack": wp, "wq2": wq2}


def make_in_maps(x, Wqkv, bqkv, Wff, bff):
    x = np.asarray(x, np.float32)
    w = _host_weights(Wqkv, bqkv, Wff, bff)
    x16 = x.astype(np.float16)
    in_maps = []
    for c in range(N_CORES):
        b, h = divmod(c, 2)
        mine = x16[b, h * NH : (h + 1) * NH]
        other = x16[b, (1 - h) * NH : (2 - h) * NH]
        xa = np.ones((NH, EA), np.float16)
        xa[:, 0:E] = mine
        xbm = np.ones((NH, EA), np.float16)
        xbm[:, 0:E] = other
        m = {"xa": xa, "xb": xbm.astype(NP_F8)}
        m.update(w)
        in_maps.append(m)
    return in_maps


def assemble(results):
    out = np.empty((B, N, E), np.float32)
    for c in range(N_CORES):
        b, h = divmod(c, 2)
        half = np.empty((NH, E), np.float32)
        o0 = results[c]["out0"].reshape(128, 4, E)
        o1 = results[c]["out1"].reshape(128, 4, E)
        half.reshape(128, 8, E)[:, 0:4] = o0
        half.reshape(128, 8, E)[:, 4:8] = o1
        out[b, h * NH : (h + 1) * NH] = half
    return out


def kernel(x, Wqkv, bqkv, Wff, bff):
    global LAST_RESULTS
    nc = get_nc()
    in_maps = make_in_maps(x, Wqkv, bqkv, Wff, bff)
    res = bass_utils.run_bass_kernel_spmd(
        nc, in_maps, core_ids=list(range(N_CORES))
    )
    LAST_RESULTS = res
    return assemble(res.results)


# revision 36
# speedup vs baseline: 1.0396x; 1.0250x over previous
"""Trainium2 Bass kernel for nn_MultiHeadAttention_7584912245188.

Reference computes (no softmax!):
    qkv = x @ Wqkv + bqkv ; split q,k,v ; per head: y = (q k^T / sqrt(D)) v
    out = y @ Wff + bff

No softmax => attention is linear and reassociates: (Q K^T) V = Q (K^T V).
With X_aug = [X | 1] ([N, 97]) and G = X_aug^T X_aug ([97, 97]) the module
collapses to out = X_aug @ Wfin computed on device as:
    V = G @ Wk_aug                                  [97, 96]  (1 matmul)
    Call_h = V_h^T @ Wvff_h                         [16, 96]  (6 matmuls,
             head blocks along the free dim; Wvff_h = D^-0.5 Wv_aug_h Wff_h)
    Wfin = sum_h Wq_aug_h @ Call_h + e_last bff^T   [97, 96]  (7-matmul group)
    out rows {8p+j} = X_chunk @ Wfin                (8 matmuls via transposed X)
O(N*E^2) instead of O(N^2*D).

Sharding (8 cores): core c -> (batch b = c//2, half h = c%2). Each core
computes the full-batch Gram redundantly (cheaper than a collective) and
writes its own half of the rows.

Schedule (fixed DMA costs dominate: HWDGE issue ~630 + 650 DGE delay, SWDGE
prep ~1040 + 650, 900ns completion sem, shared ~360GB/s DMA engines):
  - near half rides the first sync-HWDGE slot in fp16 (Gram + transposes);
    the far half is QUANTIZED TO FP8 (it only feeds the Gram; G is
    diagonally dominated, measured end-to-end rel err ~6e-3) and rides the
    Pool SWDGE path whose descriptor prep overlaps the first transfer.
  - folded weights are ~160KB (vs 262KB for the P/Q form) on the two
    activation-queue HWDGE slots, ordered Wk|Wvff|identity first.
  - a chain of tiny matmuls warms the PE p-state ramp before the real work.
  - X^T comes from 8 PE transposes placed in PE idle gaps; the PSUM->SBUF
    staging copies run on the Activation engine, off the critical chain.
  - outputs leave as two HWDGE stores (sync + activation queues) so the
    second issue/delay overlaps the first transfer.
"""

import numpy as np
from contextlib import ExitStack

import ml_dtypes
import concourse.bass as bass
import concourse.tile as tile
from concourse import bacc, mybir
from concourse import bass_utils

B, N, E = 4, 2048, 96
H = 6
D = E // H            # 16
EA = E + 1            # 97 (augmented ones column)
NH = N // 2           # 1024 rows per half
NCH = 8               # row chunks per half (chunk j = rows {8p + j})
SCALE = float(D) ** -0.5
F32 = mybir.dt.float32
F16 = mybir.dt.float16
F8 = mybir.dt.float8e4
NP_F8 = ml_dtypes.float8_e4m3

N_WARM = 12           # PE p-state warmup matmuls
WARM_COLS = 128

# wpack (fp16, 97 partitions) column layout: Wk_aug | Wvff | identity
C_WK = 0              # Wk_aug [97, 96]
C_WVFF = 96           # Wvff (6 heads x [97, 96], scale folded)
C_ID = 672            # [97, 128] f16 identity (PE transpose operand)
WCOLS = 800
# wq2 (fp16, 16 partitions): per-head Wq_aug_h^T [16, 97] | onehot | bff
C_OH = 582            # [1, 97] onehot row (1.0 at col 96) - bff placement
C_BF = 679            # [1, 96] bff row
WQCOLS = 775

N_CORES = 8

_NC_CACHE = {}
LAST_RESULTS = None


def _build_nc():
    nc = bacc.Bacc(
        "TRN2", target_bir_lowering=False, debug=False, num_devices=N_CORES,
    )
    xa = nc.dram_tensor("xa", [NH, EA], F16, kind="ExternalInput").ap()
    xb = nc.dram_tensor("xb", [NH, EA], F8, kind="ExternalInput").ap()
    wpi = nc.dram_tensor("wpack", [128, WCOLS], F16, kind="ExternalInput").ap()
    wqi = nc.dram_tensor("wq2", [D, WQCOLS], F16, kind="ExternalInput").ap()
    outd = nc.dram_tensor("out", [128, 8 * E], F16, kind="ExternalOutput").ap()

    with tile.TileContext(nc) as tc, ExitStack() as ctx:
        consts = ctx.enter_context(tc.tile_pool(name="consts", bufs=1))
        big = ctx.enter_context(tc.tile_pool(name="big", bufs=1))
        small = ctx.enter_context(tc.tile_pool(name="small", bufs=1))
        outp = ctx.enter_context(tc.tile_pool(name="outp", bufs=1))
        ps_gw = ctx.enter_context(tc.tile_pool(name="ps_gw", bufs=1, space="PSUM"))
        ps_v = ctx.enter_context(tc.tile_pool(name="ps_v", bufs=1, space="PSUM"))
        ps_c = ctx.enter_context(tc.tile_pool(name="ps_c", bufs=2, space="PSUM"))
        ps_t = ctx.enter_context(tc.tile_pool(name="ps_t", bufs=2, space="PSUM"))
        ps_o = ctx.enter_context(tc.tile_pool(name="ps_o", bufs=2, space="PSUM"))

        # --- near half fp16 on the first sync-HWDGE slot; far half fp8 on
        # the Pool SWDGE path (its prep overlaps xa's transfer); weights on
        # the activation HWDGE queue, Wk|Wvff|identity first
        XA = big.tile([128, NCH, EA], F16)
        nc.sync.dma_start(out=XA[:], in_=xa.rearrange("(p j) e -> p j e", j=NCH))
        XB = big.tile([128, NCH, EA], F8)
        nc.gpsimd.dma_start(out=XB[:], in_=xb.rearrange("(p j) e -> p j e", j=NCH))
        wp = consts.tile([128, WCOLS], F16)
        nc.scalar.dma_start(out=wp[:], in_=wpi)
        wq2 = consts.tile([D, WQCOLS], F16)
        nc.scalar.dma_start(out=wq2[:], in_=wqi)

        # --- PE p-state warmup: keep the tensor engine busy from ~0.7us so
        # the ramp model is past the slow state when the real matmuls start
        wu = small.tile([1, WARM_COLS], F16)
        nc.vector.memset(wu[:], 0.0)
        wu_ps = ps_o.tile([1, WARM_COLS], F32, tag="og", name="warm")
        for _ in range(N_WARM):
            nc.tensor.matmul(
                wu_ps[:], lhsT=wu[0:1, 0:1], rhs=wu[:], start=True, stop=True
            )

        # --- G = X_aug^T X_aug, one 16-matmul PSUM accumulation group
        # (near half first - it arrives first)
        g_ps = ps_gw.tile([EA, EA], F32, tag="gw", name="g")
        for c in range(NCH):
            xc = XA[:, c, :]
            nc.tensor.matmul(g_ps[:], lhsT=xc, rhs=xc, start=(c == 0), stop=False)
        for c in range(NCH):
            xc = XB[:, c, :]
            nc.tensor.matmul(
                g_ps[:], lhsT=xc, rhs=xc, start=False, stop=(c == NCH - 1)
            )
        g_h = small.tile([EA, EA], F16)
        nc.vector.tensor_copy(out=g_h[:], in_=g_ps[:])

        # --- 8 PE transposes of the near half (PE is idle while the chain
        # copies run); PSUM->SBUF staging on Act, off the critical chain
        XT = big.tile([EA, NCH, 128], F16)
        pts = []
        for grp in range(2):
            pt = ps_t.tile([EA, 4, 128], F16, tag="pt", name=f"pt{grp}")
            for j in range(4):
                nc.tensor.transpose(
                    out=pt[:, j, :], in_=XA[:, 4 * grp + j, :],
                    identity=wp[:, C_ID : C_ID + 128],
                )
            pts.append(pt)

        # --- V = G @ Wk_aug
        v_ps = ps_v.tile([EA, E], F32)
        nc.tensor.matmul(
            v_ps[:], lhsT=g_h[:], rhs=wp[0:EA, C_WK : C_WK + E], start=True, stop=True
        )
        v_h = small.tile([EA, E], F16)
        nc.vector.tensor_copy(out=v_h[:], in_=v_ps[:])
        for grp in range(2):
            nc.scalar.copy(
                out=XT[:, 4 * grp : 4 * (grp + 1), :], in_=pts[grp][:]
            )

        # --- Call[0:16, 96h:96h+96] = V_h^T Wvff_h (PSUM base-partition rule
        # forces head blocks onto the free dim; two banks, DVE + Act copies)
        call_sb = small.tile([D, H * E], F16)
        for half in range(2):
            ca_ps = ps_c.tile([D, 3 * E], F32, tag="call", name=f"call{half}")
            for hh in range(3):
                h = 3 * half + hh
                nc.tensor.matmul(
                    ca_ps[:, E * hh : E * (hh + 1)],
                    lhsT=v_h[:, D * h : D * (h + 1)],
                    rhs=wp[0:EA, C_WVFF + E * h : C_WVFF + E * (h + 1)],
                    start=True, stop=True,
                )
            cp = nc.vector.tensor_copy if half == 0 else nc.scalar.copy
            cp(out=call_sb[:, 3 * E * half : 3 * E * (half + 1)], in_=ca_ps[:])

        # --- Wfin = sum_h Wq_aug_h @ Call_h + e_last bff^T (one accum group,
        # PSUM bank shared with G - dead after g_h)
        wf_ps = ps_gw.tile([EA, E], F32, tag="gw", name="wf")
        for h in range(H):
            nc.tensor.matmul(
                wf_ps[:],
                lhsT=wq2[:, EA * h : EA * (h + 1)],
                rhs=call_sb[:, E * h : E * (h + 1)],
                start=(h == 0), stop=False,
            )
        nc.tensor.matmul(
            wf_ps[:],
            lhsT=wq2[0:1, C_OH : C_OH + EA],
            rhs=wq2[0:1, C_BF : C_BF + E],
            start=False, stop=True,
        )
        wf_h = small.tile([EA, E], F16)
        nc.vector.tensor_copy(out=wf_h[:], in_=wf_ps[:])

        # --- finals: out rows {8p+j} = X_chunk @ Wfin; each half leaves as
        # its own HWDGE store so the issues/transfers overlap
        osb = outp.tile([128, 2, 4 * E], F16)
        for g in range(2):
            og = ps_o.tile([128, 4, E], F32, tag="og", name=f"og{g}")
            for j4 in range(4):
                nc.tensor.matmul(
                    og[:, j4, :], lhsT=XT[:, 4 * g + j4, :], rhs=wf_h[:],
                    start=True, stop=True,
                )
            cp = nc.vector.tensor_copy if g == 0 else nc.scalar.copy
            cp(out=osb[:, g, :], in_=og[:].rearrange("p a b -> p (a b)"))
        nc.sync.dma_start(out=outd, in_=osb[:].rearrange("p a b -> p (a b)"))

    nc.compile()
    return nc


def get_nc():
    if "nc" not in _NC_CACHE:
        _NC_CACHE["nc"] = _build_nc()
    return _NC_CACHE["nc"]


def _host_weights(Wqkv, bqkv, Wff, bff):
    waug = np.concatenate(
        [np.asarray(Wqkv, np.float64), np.asarray(bqkv, np.float64)[None, :]], axis=0
    )
    Wq, Wk, Wv = waug[:, 0:E], waug[:, E : 2 * E], waug[:, 2 * E : 3 * E]
    Wff = np.asarray(Wff, np.float64)
    wp = np.zeros((128, WCOLS), np.float16)
    wp[0:EA, C_WK : C_WK + E] = Wk.astype(np.float16)
    wp[:, C_ID : C_ID + 128] = np.eye(128, dtype=np.float16)
    wq2 = np.zeros((D, WQCOLS), np.float16)
    for h in range(H):
        hd = slice(h * D, (h + 1) * D)
        wp[0:EA, C_WVFF + E * h : C_WVFF + E * (h + 1)] = (
            SCALE * (Wv[:, hd] @ Wff[hd, :])
        ).astype(np.float16)
        wq2[:, EA * h : EA * (h + 1)] = Wq[:, hd].T.astype(np.float16)
    wq2[0, C_OH + E] = 1.0
    wq2[0, C_BF : C_BF + E] = np.asarray(bff, np.float16)
    return {"wpack": wp, "wq2": wq2}


def make_in_maps(x, Wqkv, bqkv, Wff, bff):
    x = np.asarray(x, np.float32)
    w = _host_weights(Wqkv, bqkv, Wff, bff)
    x16 = x.astype(np.float16)
    in_maps = []
    for c in range(N_CORES):
        b, h = divmod(c, 2)
        mine = x16[b, h * NH : (h + 1) * NH]
        other = x16[b, (1 - h) * NH : (2 - h) * NH]
        xa = np.ones((NH, EA), np.float16)
        xa[:, 0:E] = mine
        xbm = np.ones((NH, EA), np.float16)
        xbm[:, 0:E] = other
        m = {"xa": xa, "xb": xbm.astype(NP_F8)}
        m.update(w)
        in_maps.append(m)
    return in_maps


def assemble(results):
    out = np.empty((B, N, E), np.float32)
    for c in range(N_CORES):
        b, h = divmod(c, 2)
        half = results[c]["out"].reshape(128, 8, E).astype(np.float32)
        out[b, h * NH : (h + 1) * NH] = half.reshape(NH, E)
    return out


def kernel(x, Wqkv, bqkv, Wff, bff):
    global LAST_RESULTS
    nc = get_nc()
    in_maps = make_in_maps(x, Wqkv, bqkv, Wff, bff)
    res = bass_utils.run_bass_kernel_spmd(
        nc, in_maps, core_ids=list(range(N_CORES))
    )
    LAST_RESULTS = res
    return assemble(res.results)
